# revision 8
# baseline (speedup 1.0000x reference)
"""Trainium2 Bass kernel for nn_SSMLayer_17514876633683.

Math: the reference SSM state update broadcasts the input over H and starts
from zero state, so state[b,:,h] is identical for every h.  The whole layer
collapses to:
    z_t[b]    = A @ z_{t-1}[b] + B @ x[b,t]          (z in R^S, S=128)
    c[b,t]    = Cbar . z_t[b]                         (Cbar = C.mean(0))
    y_pre     = c[b,t] + (x @ D.T)[b,t,:]
    y         = LN(gelu(y_pre) + x) * gamma + beta

Sharding: 8 cores = 4 batches x 2 time-halves.  Every core runs the same
SPMD program: "scan all 512 steps of the provided x, output rows 256..511".
The first-half core of each batch receives x zero-padded at the front so its
output rows land in [256, 512) too.

Scan mapping on device (per core, its batch):
  U = B @ x^T                               (S x T)       - PE matmuls
  R_j = sum_r A^(Q-1-r) U[:, jQ+r]          (chunk summaries, Q=16, 32 chunks)
  Z_j = sum_{j'<j} (A^Q)^(j-1-j') R_j'      (chunk-boundary states; 31 lag
                                             matmuls with precomputed powers,
                                             no sequential round-trips)
  c[jQ+i] = g_i . Z_j + sum_{k<i} g_{i-1-k} . U[:, jQ+k]   (g_k = (A^T)^k Cbar)
All A-power / g weight matrices are precomputed host-side from the inputs.
"""

import sys
from contextlib import ExitStack

sys.path.insert(0, "/opt/trn_rl_repo")

import numpy as np

import concourse.bass as bass  # noqa: F401  (import keeps bass registered)
import concourse.mybir as mybir
import concourse.tile as tile
from concourse import bacc, bass_utils

# Problem shapes (hardcoded per the harness contract).
BSZ, T, H, S = 4, 512, 512, 128
Q = 16           # scan chunk length
NCH = T // Q     # 32 chunks
NLAG = NCH - 1   # 31 boundary lags
TOUT = 256       # output rows per core
LN_EPS = 1e-5
NCORES = 8

F32 = mybir.dt.float32
AF = mybir.ActivationFunctionType


def _host_weights(A, Bm, Cm):
    """Precompute the scan weight matrices (float64 for the matrix powers)."""
    A64 = A.astype(np.float64)
    Cbar = Cm.astype(np.float64).mean(axis=0)          # (S,)

    pows = [np.eye(S)]
    for _ in range(1, Q + 1):
        pows.append(pows[-1] @ A64)                    # pows[k] = A^k
    A16 = pows[Q]

    # lhsT tiles for R: column block r holds (A^(Q-1-r))^T
    APOW = np.concatenate([pows[Q - 1 - r].T for r in range(Q)], axis=1)

    q16 = [np.eye(S)]
    for _ in range(1, NLAG):
        q16.append(q16[-1] @ A16)                      # (A^Q)^L
    APQL = np.concatenate([q16[L].T for L in range(NLAG)], axis=1)

    g = [pows[k].T @ Cbar for k in range(Q)]           # g_k = (A^T)^k Cbar
    G16 = np.stack(g, axis=1)                          # (S, Q)
    WTRI = np.zeros((S, Q * Q))
    for k in range(Q):
        for i in range(Q):
            if i > k:
                WTRI[:, k * Q + i] = g[i - 1 - k]
    GW = np.concatenate([G16, WTRI], axis=1)           # (S, Q + Q*Q)

    return (
        APOW.astype(np.float32),
        APQL.astype(np.float32),
        GW.astype(np.float32),
    )


def _emit(tc, aps, apply_gamma_beta):
    nc = tc.nc
    xb, Bt, Dt, APOW, APQL, GW, ident, yout = (
        aps["xb"], aps["Bt"], aps["Dt"], aps["APOW"], aps["APQL"],
        aps["GW"], aps["ident"], aps["yout"],
    )

    ctx = ExitStack()
    cpool = ctx.enter_context(tc.tile_pool(name="const", bufs=1))
    wpool = ctx.enter_context(tc.tile_pool(name="work", bufs=2))
    tpp = ctx.enter_context(tc.tile_pool(name="tpp", bufs=2, space="PSUM"))
    spp = ctx.enter_context(tc.tile_pool(name="spp", bufs=1, space="PSUM"))
    ypp = ctx.enter_context(tc.tile_pool(name="ypp", bufs=2, space="PSUM"))

    # ---- input loads -------------------------------------------------------
    xb_sb = cpool.tile([128, 4, H], F32, tag="xb_sb")
    nc.sync.dma_start(xb_sb[:], xb.rearrange("(tt p) h -> p tt h", p=128))
    Bt_sb = cpool.tile([128, 4, S], F32, tag="Bt_sb")
    nc.sync.dma_start(Bt_sb[:], Bt.rearrange("(hh p) s -> p hh s", p=128))
    Dt_sb = cpool.tile([128, 4, H], F32, tag="Dt_sb")
    nc.sync.dma_start(Dt_sb[:], Dt.rearrange("(hh p) o -> p hh o", p=128))
    APOW_sb = cpool.tile([128, Q * S], F32, tag="APOW_sb")
    nc.sync.dma_start(APOW_sb[:], APOW)
    APQL_sb = cpool.tile([128, NLAG * S], F32, tag="APQL_sb")
    nc.sync.dma_start(APQL_sb[:], APQL)
    GW_sb = cpool.tile([128, Q + Q * Q], F32, tag="GW_sb")
    nc.sync.dma_start(GW_sb[:], GW)
    id_sb = cpool.tile([128, 128], F32, tag="id_sb")
    nc.sync.dma_start(id_sb[:], ident)
    eps_sb = cpool.tile([128, 1], F32, tag="eps_sb")
    nc.gpsimd.memset(eps_sb[:], LN_EPS)
    if apply_gamma_beta:
        gb_sb = cpool.tile([128, 2, H], F32, tag="gb_sb")
        nc.sync.dma_start(gb_sb[:], aps["gb"].rearrange("(u p) g h -> p g h", p=128))

    # ---- transpose x: xT[hh] is (h-part x t-free) --------------------------
    xT = [cpool.tile([128, T], F32, tag=f"xT{hh}", name=f"xT{hh}")
          for hh in range(4)]
    for hh in range(4):
        for tt in range(4):
            pt = tpp.tile([128, 128], F32, tag="tp")
            nc.tensor.transpose(pt[:], xb_sb[:, tt, hh * 128:(hh + 1) * 128], id_sb[:])
            dst = xT[hh][:, tt * 128:(tt + 1) * 128]
            if (hh * 4 + tt) % 2 == 0:
                nc.vector.tensor_copy(dst, pt[:])
            else:
                nc.scalar.copy(dst, pt[:])

    # ---- U = B @ x^T  (S x T) ---------------------------------------------
    U_ps = spp.tile([128, T], F32, tag="U_ps")
    for hh in range(4):
        nc.tensor.matmul(U_ps[:], lhsT=Bt_sb[:, hh, :], rhs=xT[hh][:],
                         start=(hh == 0), stop=(hh == 3))
    U_sb = cpool.tile([128, T], F32, tag="U_sb")
    nc.vector.tensor_copy(U_sb[:], U_ps[:])
    U_r = U_sb.rearrange("s (j r) -> s r j", r=Q)      # [128, Q, NCH]

    # ---- chunk summaries R ------------------------------------------------
    R_ps = spp.tile([128, NCH], F32, tag="scan_ps")
    for r in range(Q):
        nc.tensor.matmul(R_ps[:], lhsT=APOW_sb[:, r * S:(r + 1) * S],
                         rhs=U_r[:, r, :], start=(r == 0), stop=(r == Q - 1))
    R_sb = cpool.tile([128, NCH], F32, tag="R_sb")
    nc.vector.tensor_copy(R_sb[:], R_ps[:])

    # ---- boundary states Z (block-Toeplitz matmuls over lags) -------------
    Z_ps = spp.tile([128, NCH], F32, tag="scan_ps")
    for L in range(NLAG):
        nc.tensor.matmul(Z_ps[:, L + 1:NCH], lhsT=APQL_sb[:, L * S:(L + 1) * S],
                         rhs=R_sb[:, 0:NCH - 1 - L],
                         start=(L == 0), stop=(L == NLAG - 1))
    Z_sb = cpool.tile([128, NCH], F32, tag="Z_sb")
    nc.any.memzero(Z_sb[:])
    nc.vector.tensor_copy(Z_sb[:, 1:NCH], Z_ps[:, 1:NCH])

    # ---- c = G^T Z + triangular intra-chunk term --------------------------
    c_ps = spp.tile([16, NCH], F32, tag="scan_ps")
    nc.tensor.matmul(c_ps[:], lhsT=GW_sb[:, 0:Q], rhs=Z_sb[:],
                     start=True, stop=False)
    for k in range(Q):
        nc.tensor.matmul(c_ps[:], lhsT=GW_sb[:, Q + k * Q:Q + (k + 1) * Q],
                         rhs=U_r[:, k, :], start=False, stop=(k == Q - 1))
    c_sb = cpool.tile([16, NCH], F32, tag="c_sb")
    nc.vector.tensor_copy(c_sb[:], c_ps[:])

    # ---- reshape c (i x j) -> per-row column for the output half ----------
    # t = j*Q + i ; output rows are t in [256, 512) -> j in [16, 32)
    # partition p = t - 256 = (j%8)*16 + i, free n = (j-16)//8
    c_col = cpool.tile([128, 2], F32, tag="c_col")
    c_r = c_sb.rearrange("i (n jm) -> i n jm", jm=8)
    for jm in range(8):
        nc.sync.dma_start(c_col[jm * 16:(jm + 1) * 16, :], c_r[:, 2:4, jm])

    # ---- xD + gelu + residual + layernorm ---------------------------------
    for tt2 in range(2):
        y_ps = ypp.tile([128, H], F32, tag="y_ps")
        for hh in range(4):
            nc.tensor.matmul(
                y_ps[:],
                lhsT=xT[hh][:, 256 + tt2 * 128:256 + (tt2 + 1) * 128],
                rhs=Dt_sb[:, hh, :], start=(hh == 0), stop=(hh == 3))
        g_sb = wpool.tile([128, H], F32, tag="g_sb")
        nc.scalar.activation(g_sb[:], y_ps[:], AF.Gelu,
                             bias=c_col[:, tt2:tt2 + 1], scale=1.0)
        y_sb = wpool.tile([128, H], F32, tag="y_sb")
        nc.vector.tensor_add(y_sb[:], g_sb[:], xb_sb[:, 2 + tt2, :])
        st6 = wpool.tile([128, 6], F32, tag="st6")
        nc.vector.bn_stats(st6[:], y_sb[:])
        mv = wpool.tile([128, 2], F32, tag="mv")
        nc.vector.bn_aggr(mv[:], st6[:])
        sd = wpool.tile([128, 1], F32, tag="sd")
        nc.scalar.activation(sd[:], mv[:, 1:2], AF.Sqrt, bias=eps_sb[:], scale=1.0)
        iv = wpool.tile([128, 1], F32, tag="iv")
        nc.vector.reciprocal(iv[:], sd[:])
        o_sb = wpool.tile([128, H], F32, tag="o_sb")
        nc.vector.tensor_scalar(o_sb[:], y_sb[:], mv[:, 0:1], iv[:],
                                op0=mybir.AluOpType.subtract,
                                op1=mybir.AluOpType.mult)
        if apply_gamma_beta:
            nc.vector.tensor_mul(o_sb[:], o_sb[:], gb_sb[:, 0, :])
            nc.vector.tensor_add(o_sb[:], o_sb[:], gb_sb[:, 1, :])
        nc.sync.dma_start(yout[tt2 * 128:(tt2 + 1) * 128, :], o_sb[:])

    ctx.close()


def _build_program(apply_gamma_beta):
    nc = bacc.Bacc("TRN2", target_bir_lowering=False, debug=False,
                   enable_asserts=False, num_devices=NCORES)
    aps = {
        "xb": nc.dram_tensor("xb", (T, H), F32, kind="ExternalInput").ap(),
        "Bt": nc.dram_tensor("Bt", (H, S), F32, kind="ExternalInput").ap(),
        "Dt": nc.dram_tensor("Dt", (H, H), F32, kind="ExternalInput").ap(),
        "APOW": nc.dram_tensor("APOW", (S, Q * S), F32, kind="ExternalInput").ap(),
        "APQL": nc.dram_tensor("APQL", (S, NLAG * S), F32, kind="ExternalInput").ap(),
        "GW": nc.dram_tensor("GW", (S, Q + Q * Q), F32, kind="ExternalInput").ap(),
        "ident": nc.dram_tensor("ident", (128, 128), F32, kind="ExternalInput").ap(),
        "yout": nc.dram_tensor("yout", (TOUT, H), F32, kind="ExternalOutput").ap(),
    }
    if apply_gamma_beta:
        aps["gb"] = nc.dram_tensor("gb", (128, 2, H), F32, kind="ExternalInput").ap()
    with tile.TileContext(nc) as tc:
        _emit(tc, aps, apply_gamma_beta)
    nc.compile()
    return nc


def _prepare_in_maps(x, A, Bm, Cm, D, gamma, beta, apply_gamma_beta):
    APOW, APQL, GW = _host_weights(A, Bm, Cm)
    base = {
        "Bt": np.ascontiguousarray(Bm.T.astype(np.float32)),
        "Dt": np.ascontiguousarray(D.T.astype(np.float32)),
        "APOW": APOW,
        "APQL": APQL,
        "GW": GW,
        "ident": np.eye(128, dtype=np.float32),
    }
    if apply_gamma_beta:
        gb = np.stack([np.broadcast_to(gamma, (128, H)),
                       np.broadcast_to(beta, (128, H))], axis=1)
        base["gb"] = np.ascontiguousarray(gb.astype(np.float32))
    in_maps = []
    for core in range(NCORES):
        b, half = core // 2, core % 2
        if half == 0:
            xb = np.concatenate(
                [np.zeros((TOUT, H), np.float32), x[b, :TOUT]], axis=0)
        else:
            xb = x[b]
        in_maps.append({**base, "xb": np.ascontiguousarray(xb.astype(np.float32))})
    return in_maps


def _run(inputs, trace=False):
    x = np.asarray(inputs["x"], np.float32)
    A = np.asarray(inputs["A"], np.float32)
    Bm = np.asarray(inputs["B"], np.float32)
    Cm = np.asarray(inputs["C"], np.float32)
    D = np.asarray(inputs["D"], np.float32)
    gamma = np.asarray(inputs["gamma"], np.float32)
    beta = np.asarray(inputs["beta"], np.float32)

    apply_gamma_beta = not (np.all(gamma == 1.0) and np.all(beta == 0.0))
    nc = _build_program(apply_gamma_beta)
    in_maps = _prepare_in_maps(x, A, Bm, Cm, D, gamma, beta, apply_gamma_beta)
    res = bass_utils.run_bass_kernel_spmd(
        nc, in_maps, core_ids=list(range(NCORES)), trace=trace)
    y = np.empty((BSZ, T, H), np.float32)
    for core in range(NCORES):
        b, half = core // 2, core % 2
        y[b, half * TOUT:(half + 1) * TOUT, :] = res.results[core]["yout"]
    return y, res


def kernel(**inputs):
    y, _ = _run(inputs, trace=False)
    return y


def kernel_traced(**inputs):
    return _run(inputs, trace=True)


# revision 10
# speedup vs baseline: 1.7776x; 1.7776x over previous
"""Trainium2 Bass kernel for nn_SSMLayer_17514876633683.

Math: the reference SSM state update broadcasts the input over H and starts
from zero state, so state[b,:,h] is identical for every h.  The whole layer
collapses to:
    z_t[b]    = A @ z_{t-1}[b] + B @ x[b,t]          (z in R^S, S=128)
    c[b,t]    = Cbar . z_t[b]                         (Cbar = C.mean(0))
    y_pre     = c[b,t] + (x @ D.T)[b,t,:]
    y         = LN(gelu(y_pre) + x) * gamma + beta

Sharding: 8 cores = 4 batches x 2 time-halves.  Every core runs the same
SPMD program: "scan all 512 steps of the provided x, output rows 256..511".
The first-half core of each batch receives x zero-padded at the front so its
output rows land in [256, 512) too.

Scan mapping on device (per core, its batch):
  U = B @ x^T                               (S x T)       - PE matmuls
  R_j = sum_r A^(Q-1-r) U[:, jQ+r]          (chunk summaries, Q=16, 32 chunks)
  Z_j = sum_{L<LZ} (A^Q)^L R_{j-1-L}        (chunk-boundary states; LZ lag
                                             matmuls with precomputed powers -
                                             higher lags are dropped when
                                             ||(A^Q)^L|| is negligible)
  c[jQ+i] = g_i . Z_j + sum_{k<i} g_{i-1-k} . U[:, jQ+k]   (g_k = (A^T)^k Cbar)
All A-power / g weight matrices are precomputed host-side from the inputs.
Matmul operands are bf16 (fp32 PSUM accumulation); the residual/layernorm
path stays fp32.
"""

import sys
from contextlib import ExitStack

sys.path.insert(0, "/opt/trn_rl_repo")

import ml_dtypes
import numpy as np

import concourse.bass as bass  # noqa: F401
import concourse.mybir as mybir
import concourse.tile as tile
from concourse import bacc, bass_utils

# Problem shapes (hardcoded per the harness contract).
BSZ, T, H, S = 4, 512, 512, 128
Q = 16           # scan chunk length
NCH = T // Q     # 32 chunks
TOUT = 256       # output rows per core
LN_EPS = 1e-5
NCORES = 8
NWARM = 18       # PE warmup matmuls (~3.6us busy to trip the HAM un-throttle)
TRUNC_TOL = 1e-5

F32 = mybir.dt.float32
BF16 = mybir.dt.bfloat16
BF16_NP = ml_dtypes.bfloat16
AF = mybir.ActivationFunctionType


def _host_weights(A, Bm, Cm):
    """Precompute scan weights; returns (APOW, APQL, GW, LZ)."""
    A64 = A.astype(np.float64)
    Cbar = Cm.astype(np.float64).mean(axis=0)          # (S,)

    pows = [np.eye(S)]
    for _ in range(Q):
        pows.append(pows[-1] @ A64)                    # pows[k] = A^k
    A16 = pows[Q]

    # lhsT tiles for R: column block r holds (A^(Q-1-r))^T
    APOW = np.concatenate([pows[Q - 1 - r].T for r in range(Q)], axis=1)

    # boundary-lag powers, truncated once ||(A^Q)^L|| is negligible
    q16 = [np.eye(S)]
    while len(q16) < NCH - 1:
        nxt = q16[-1] @ A16
        if np.linalg.norm(nxt, 2) < TRUNC_TOL:
            break
        q16.append(nxt)
    LZ = len(q16)
    APQL = np.concatenate([m.T for m in q16], axis=1)

    g = [pows[k].T @ Cbar for k in range(Q)]           # g_k = (A^T)^k Cbar
    G16 = np.stack(g, axis=1)                          # (S, Q)
    WTRI = np.zeros((S, Q * Q))
    for k in range(Q):
        for i in range(Q):
            if i > k:
                WTRI[:, k * Q + i] = g[i - 1 - k]
    GW = np.concatenate([G16, WTRI], axis=1)           # (S, Q + Q*Q)

    return (
        APOW.astype(BF16_NP),
        APQL.astype(BF16_NP),
        GW.astype(BF16_NP),
        LZ,
    )


def _emit(tc, aps, apply_gamma_beta, LZ):
    nc = tc.nc
    xb, xres, Bt, Dt, APOW, APQL, GW, yout = (
        aps["xb"], aps["xres"], aps["Bt"], aps["Dt"], aps["APOW"],
        aps["APQL"], aps["GW"], aps["yout"],
    )

    ctx = ExitStack()
    cpool = ctx.enter_context(tc.tile_pool(name="const", bufs=1))
    wpool = ctx.enter_context(tc.tile_pool(name="work", bufs=2))
    spp = ctx.enter_context(tc.tile_pool(name="spp", bufs=1, space="PSUM"))
    ypp = ctx.enter_context(tc.tile_pool(name="ypp", bufs=2, space="PSUM"))
    wmp = ctx.enter_context(tc.tile_pool(name="wmp", bufs=1, space="PSUM"))

    # ---- PE warmup + gelu table preload (runs while input DMAs land) ------
    warm_sb = cpool.tile([128, 512], BF16, tag="warm_sb")
    nc.gpsimd.memset(warm_sb[:], 0.0)
    for i in range(NWARM):
        wp = wmp.tile([128, 512], F32, tag="warm_ps", name=f"wp{i}")
        nc.tensor.matmul(wp[:], lhsT=warm_sb[:, :128], rhs=warm_sb[:],
                         start=True, stop=True)
    gsc = cpool.tile([128, 1], F32, tag="gsc")
    nc.gpsimd.memset(gsc[:], 0.0)
    nc.scalar.activation(gsc[:], gsc[:], AF.Gelu)

    eps_sb = cpool.tile([128, 1], F32, tag="eps_sb")
    nc.gpsimd.memset(eps_sb[:], LN_EPS)

    # ---- input loads -------------------------------------------------------
    # x^T via DMA transpose (bf16): xT[hh] is (h-part x t-free)
    xT = [cpool.tile([128, T], BF16, tag=f"xT{hh}", name=f"xT{hh}")
          for hh in range(4)]
    for hh in range(4):
        nc.sync.dma_start_transpose(xT[hh][:], xb[:, hh * 128:(hh + 1) * 128])
    xres_sb = cpool.tile([128, 2, H], F32, tag="xres_sb")
    nc.sync.dma_start(xres_sb[:], xres.rearrange("(tt p) h -> p tt h", p=128))
    Bt_sb = cpool.tile([128, 4, S], BF16, tag="Bt_sb")
    nc.sync.dma_start(Bt_sb[:], Bt.rearrange("(hh p) s -> p hh s", p=128))
    Dt_sb = cpool.tile([128, 4, H], BF16, tag="Dt_sb")
    nc.sync.dma_start(Dt_sb[:], Dt.rearrange("(hh p) o -> p hh o", p=128))
    APOW_sb = cpool.tile([128, Q * S], BF16, tag="APOW_sb")
    nc.sync.dma_start(APOW_sb[:], APOW)
    APQL_sb = cpool.tile([128, LZ * S], BF16, tag="APQL_sb")
    nc.sync.dma_start(APQL_sb[:], APQL)
    GW_sb = cpool.tile([128, Q + Q * Q], BF16, tag="GW_sb")
    nc.sync.dma_start(GW_sb[:], GW)
    if apply_gamma_beta:
        gb_sb = cpool.tile([128, 2, H], F32, tag="gb_sb")
        nc.sync.dma_start(gb_sb[:], aps["gb"].rearrange("(u p) g h -> p g h", p=128))

    # ---- U = B @ x^T  (S x T) ---------------------------------------------
    U_ps = spp.tile([128, T], F32, tag="U_ps")
    for hh in range(4):
        nc.tensor.matmul(U_ps[:], lhsT=Bt_sb[:, hh, :], rhs=xT[hh][:],
                         start=(hh == 0), stop=(hh == 3))
    U_sb = cpool.tile([128, T], BF16, tag="U_sb")
    nc.vector.tensor_copy(U_sb[:], U_ps[:])
    U_r = U_sb.rearrange("s (j r) -> s r j", r=Q)      # [128, Q, NCH]

    # ---- chunk summaries R ------------------------------------------------
    R_ps = spp.tile([128, NCH], F32, tag="scan_ps")
    for r in range(Q):
        nc.tensor.matmul(R_ps[:], lhsT=APOW_sb[:, r * S:(r + 1) * S],
                         rhs=U_r[:, r, :], start=(r == 0), stop=(r == Q - 1))
    R_sb = cpool.tile([128, NCH], BF16, tag="R_sb")
    nc.vector.tensor_copy(R_sb[:], R_ps[:])

    # ---- boundary states Z (block-Toeplitz matmuls over lags) -------------
    Z_ps = spp.tile([128, NCH], F32, tag="scan_ps")
    for L in range(LZ):
        nc.tensor.matmul(Z_ps[:, L + 1:NCH], lhsT=APQL_sb[:, L * S:(L + 1) * S],
                         rhs=R_sb[:, 0:NCH - 1 - L],
                         start=(L == 0), stop=(L == LZ - 1))
    Z_sb = cpool.tile([128, NCH], BF16, tag="Z_sb")
    nc.any.memzero(Z_sb[:])
    nc.vector.tensor_copy(Z_sb[:, 1:NCH], Z_ps[:, 1:NCH])

    # ---- c = G^T Z + triangular intra-chunk term --------------------------
    c_ps = spp.tile([16, NCH], F32, tag="scan_ps")
    nc.tensor.matmul(c_ps[:], lhsT=GW_sb[:, 0:Q], rhs=Z_sb[:],
                     start=True, stop=False)
    for k in range(Q):
        nc.tensor.matmul(c_ps[:], lhsT=GW_sb[:, Q + k * Q:Q + (k + 1) * Q],
                         rhs=U_r[:, k, :], start=False, stop=(k == Q - 1))
    c_sb = cpool.tile([16, NCH], F32, tag="c_sb")
    nc.vector.tensor_copy(c_sb[:], c_ps[:])

    # ---- reshape c (i x j) -> per-row column for the output half ----------
    # t = j*Q + i ; output rows are t in [256, 512) -> j in [16, 32)
    # partition p = t - 256 = (j%8)*16 + i, free n = (j-16)//8
    c_col = cpool.tile([128, 2], F32, tag="c_col")
    c_r = c_sb.rearrange("i (n jm) -> i n jm", jm=8)
    for jm in range(8):
        nc.gpsimd.dma_start(c_col[jm * 16:(jm + 1) * 16, :], c_r[:, 2:4, jm])

    # ---- xD + gelu + residual + layernorm ---------------------------------
    for tt2 in range(2):
        y_ps = ypp.tile([128, H], F32, tag="y_ps")
        for hh in range(4):
            nc.tensor.matmul(
                y_ps[:],
                lhsT=xT[hh][:, 256 + tt2 * 128:256 + (tt2 + 1) * 128],
                rhs=Dt_sb[:, hh, :], start=(hh == 0), stop=(hh == 3))
        g_sb = wpool.tile([128, H], F32, tag="g_sb")
        nc.scalar.activation(g_sb[:], y_ps[:], AF.Gelu,
                             bias=c_col[:, tt2:tt2 + 1], scale=1.0)
        y_sb = wpool.tile([128, H], F32, tag="y_sb")
        nc.vector.tensor_add(y_sb[:], g_sb[:], xres_sb[:, tt2, :])
        st6 = wpool.tile([128, 6], F32, tag="st6")
        nc.vector.bn_stats(st6[:], y_sb[:])
        mv = wpool.tile([128, 2], F32, tag="mv")
        nc.vector.bn_aggr(mv[:], st6[:])
        sd = wpool.tile([128, 1], F32, tag="sd")
        nc.scalar.activation(sd[:], mv[:, 1:2], AF.Sqrt, bias=eps_sb[:], scale=1.0)
        iv = wpool.tile([128, 1], F32, tag="iv")
        nc.vector.reciprocal(iv[:], sd[:])
        o_sb = wpool.tile([128, H], F32, tag="o_sb")
        nc.vector.tensor_scalar(o_sb[:], y_sb[:], mv[:, 0:1], iv[:],
                                op0=mybir.AluOpType.subtract,
                                op1=mybir.AluOpType.mult)
        if apply_gamma_beta:
            nc.vector.tensor_mul(o_sb[:], o_sb[:], gb_sb[:, 0, :])
            nc.vector.tensor_add(o_sb[:], o_sb[:], gb_sb[:, 1, :])
        nc.sync.dma_start(yout[tt2 * 128:(tt2 + 1) * 128, :], o_sb[:])

    ctx.close()


def _build_program(apply_gamma_beta, LZ):
    nc = bacc.Bacc("TRN2", target_bir_lowering=False, debug=False,
                   enable_asserts=False, num_devices=NCORES)
    aps = {
        "xb": nc.dram_tensor("xb", (T, H), BF16, kind="ExternalInput").ap(),
        "xres": nc.dram_tensor("xres", (TOUT, H), F32, kind="ExternalInput").ap(),
        "Bt": nc.dram_tensor("Bt", (H, S), BF16, kind="ExternalInput").ap(),
        "Dt": nc.dram_tensor("Dt", (H, H), BF16, kind="ExternalInput").ap(),
        "APOW": nc.dram_tensor("APOW", (S, Q * S), BF16, kind="ExternalInput").ap(),
        "APQL": nc.dram_tensor("APQL", (S, LZ * S), BF16, kind="ExternalInput").ap(),
        "GW": nc.dram_tensor("GW", (S, Q + Q * Q), BF16, kind="ExternalInput").ap(),
        "yout": nc.dram_tensor("yout", (TOUT, H), F32, kind="ExternalOutput").ap(),
    }
    if apply_gamma_beta:
        aps["gb"] = nc.dram_tensor("gb", (128, 2, H), F32, kind="ExternalInput").ap()
    with tile.TileContext(nc) as tc:
        _emit(tc, aps, apply_gamma_beta, LZ)
    nc.compile()
    return nc


def _prepare_in_maps(x, A, Bm, Cm, D, gamma, beta, apply_gamma_beta):
    APOW, APQL, GW, LZ = _host_weights(A, Bm, Cm)
    base = {
        "Bt": np.ascontiguousarray(Bm.T).astype(BF16_NP),
        "Dt": np.ascontiguousarray(D.T).astype(BF16_NP),
        "APOW": APOW,
        "APQL": APQL,
        "GW": GW,
    }
    if apply_gamma_beta:
        gb = np.stack([np.broadcast_to(gamma, (128, H)),
                       np.broadcast_to(beta, (128, H))], axis=1)
        base["gb"] = np.ascontiguousarray(gb.astype(np.float32))
    in_maps = []
    for core in range(NCORES):
        b, half = core // 2, core % 2
        if half == 0:
            xb = np.concatenate(
                [np.zeros((TOUT, H), np.float32), x[b, :TOUT]], axis=0)
        else:
            xb = x[b]
        in_maps.append({
            **base,
            "xb": np.ascontiguousarray(xb).astype(BF16_NP),
            "xres": np.ascontiguousarray(xb[TOUT:]).astype(np.float32),
        })
    return in_maps, LZ


def _run(inputs, trace=False):
    x = np.asarray(inputs["x"], np.float32)
    A = np.asarray(inputs["A"], np.float32)
    Bm = np.asarray(inputs["B"], np.float32)
    Cm = np.asarray(inputs["C"], np.float32)
    D = np.asarray(inputs["D"], np.float32)
    gamma = np.asarray(inputs["gamma"], np.float32)
    beta = np.asarray(inputs["beta"], np.float32)

    apply_gamma_beta = not (np.all(gamma == 1.0) and np.all(beta == 0.0))
    in_maps, LZ = _prepare_in_maps(x, A, Bm, Cm, D, gamma, beta,
                                   apply_gamma_beta)
    nc = _build_program(apply_gamma_beta, LZ)
    res = bass_utils.run_bass_kernel_spmd(
        nc, in_maps, core_ids=list(range(NCORES)), trace=trace)
    y = np.empty((BSZ, T, H), np.float32)
    for core in range(NCORES):
        b, half = core // 2, core % 2
        y[b, half * TOUT:(half + 1) * TOUT, :] = res.results[core]["yout"]
    return y, res


def kernel(**inputs):
    y, _ = _run(inputs, trace=False)
    return y


def kernel_traced(**inputs):
    return _run(inputs, trace=True)


# revision 14
# speedup vs baseline: 1.9810x; 1.1144x over previous
"""Trainium2 Bass kernel for nn_SSMLayer_17514876633683.

Math: the reference SSM state update broadcasts the input over H and starts
from zero state, so state[b,:,h] is identical for every h.  The whole layer
collapses to:
    z_t[b]    = A @ z_{t-1}[b] + B @ x[b,t]          (z in R^S, S=128)
    c[b,t]    = Cbar . z_t[b]                         (Cbar = C.mean(0))
    y_pre     = c[b,t] + (x @ D.T)[b,t,:]
    y         = LN(gelu(y_pre) + x) * gamma + beta

Sharding: 8 cores = 4 batches x 2 time-halves.  Every core runs the same
SPMD program: "scan all 512 steps of the provided x, output rows 256..511".
The first-half core of each batch receives x zero-padded at the front so its
output rows land in [256, 512) too.

Scan mapping on device (per core, its batch):
  U = B @ x^T                               (S x T)       - PE matmuls
  R_j = sum_r A^(Q-1-r) U[:, jQ+r]          (chunk summaries, Q=16, 32 chunks)
  Z_j = sum_{L<LZ} (A^Q)^L R_{j-1-L}        (chunk-boundary states; LZ lag
                                             matmuls with precomputed powers -
                                             higher lags are dropped when
                                             ||(A^Q)^L|| is negligible)
  c[jQ+i] = g_i . Z_j + sum_{k<i} g_{i-1-k} . U[:, jQ+k]   (g_k = (A^T)^k Cbar)
All A-power / g weight matrices are precomputed host-side from the inputs.
Matmul operands are bf16 (fp32 PSUM accumulation); the residual/layernorm
path stays fp32.
"""

import sys
from contextlib import ExitStack

sys.path.insert(0, "/opt/trn_rl_repo")

import ml_dtypes
import numpy as np

import concourse.bass as bass  # noqa: F401
import concourse.mybir as mybir
import concourse.tile as tile
from concourse import bacc, bass_utils

# Problem shapes (hardcoded per the harness contract).
BSZ, T, H, S = 4, 512, 512, 128
Q = 16           # scan chunk length
NCH = T // Q     # 32 chunks
TOUT = 256       # output rows per core
LN_EPS = 1e-5
NCORES = 8
NWARM = 18       # PE warmup matmuls (~3.6us busy to trip the HAM un-throttle)
TRUNC_TOL = 1e-5

F32 = mybir.dt.float32
BF16 = mybir.dt.bfloat16
BF16_NP = ml_dtypes.bfloat16
AF = mybir.ActivationFunctionType


def _host_weights(A, Bm, Cm):
    """Precompute scan weights; returns (APOW, APQL, GW, LZ)."""
    A64 = A.astype(np.float64)
    Cbar = Cm.astype(np.float64).mean(axis=0)          # (S,)

    pows = [np.eye(S)]
    for _ in range(Q):
        pows.append(pows[-1] @ A64)                    # pows[k] = A^k
    A16 = pows[Q]

    # lhsT tiles for R: column block r holds (A^(Q-1-r))^T
    APOW = np.concatenate([pows[Q - 1 - r].T for r in range(Q)], axis=1)

    # boundary-lag powers, truncated once ||(A^Q)^L|| is negligible
    q16 = [np.eye(S)]
    while len(q16) < NCH - 1:
        nxt = q16[-1] @ A16
        if np.linalg.norm(nxt, 2) < TRUNC_TOL:
            break
        q16.append(nxt)
    LZ = len(q16)
    APQL = np.concatenate([m.T for m in q16], axis=1)

    g = [pows[k].T @ Cbar for k in range(Q)]           # g_k = (A^T)^k Cbar
    G16 = np.stack(g, axis=1)                          # (S, Q)
    WTRI = np.zeros((S, Q * Q))
    for k in range(Q):
        for i in range(Q):
            if i > k:
                WTRI[:, k * Q + i] = g[i - 1 - k]
    GW = np.concatenate([G16, WTRI], axis=1)           # (S, Q + Q*Q)

    return (
        APOW.astype(BF16_NP),
        APQL.astype(BF16_NP),
        GW.astype(BF16_NP),
        LZ,
    )


def _emit(tc, aps, apply_gamma_beta, LZ):
    nc = tc.nc
    xb, xres, Bt, Dt, APOW, APQL, GW, yout = (
        aps["xb"], aps["xres"], aps["Bt"], aps["Dt"], aps["APOW"],
        aps["APQL"], aps["GW"], aps["yout"],
    )

    ctx = ExitStack()
    cpool = ctx.enter_context(tc.tile_pool(name="const", bufs=1))
    wpool = ctx.enter_context(tc.tile_pool(name="work", bufs=2))
    spp = ctx.enter_context(tc.tile_pool(name="spp", bufs=1, space="PSUM"))
    ypp = ctx.enter_context(tc.tile_pool(name="ypp", bufs=2, space="PSUM"))
    wmp = ctx.enter_context(tc.tile_pool(name="wmp", bufs=1, space="PSUM"))
    dpool = ctx.enter_context(tc.tile_pool(name="dram", bufs=1, space="DRAM"))

    # ---- PE warmup + gelu table preload (runs while input DMAs land) ------
    # One accumulation group so the matmuls pipeline back-to-back and trip
    # the HAM un-throttle (isolated matmuls never warm the clock gate).
    warm_sb = cpool.tile([128, 512], BF16, tag="warm_sb")
    nc.gpsimd.memset(warm_sb[:], 0.0)
    wp = wmp.tile([128, 512], F32, tag="warm_ps")
    for i in range(NWARM):
        nc.tensor.matmul(wp[:], lhsT=warm_sb[:, :128], rhs=warm_sb[:],
                         start=(i == 0), stop=(i == NWARM - 1))
    gsc = cpool.tile([128, 1], F32, tag="gsc")
    nc.gpsimd.memset(gsc[:], 0.0)
    nc.scalar.activation(gsc[:], gsc[:], AF.Gelu)

    eps_sb = cpool.tile([128, 1], F32, tag="eps_sb")
    nc.gpsimd.memset(eps_sb[:], LN_EPS)
    big_sb = cpool.tile([128, 1], F32, tag="big_sb")
    nc.gpsimd.memset(big_sb[:], 1.0e4)

    # ---- input loads (spread across both HWDGE rings + SWDGE) -------------
    # x^T via DMA transpose (bf16): xT[hh] is (h-part x t-free)
    xT = [cpool.tile([128, T], BF16, tag=f"xT{hh}", name=f"xT{hh}")
          for hh in range(4)]
    for hh in range(4):
        eng = nc.sync if hh % 2 == 0 else nc.scalar
        eng.dma_start_transpose(xT[hh][:], xb[:, hh * 128:(hh + 1) * 128])
    xres_sb = cpool.tile([128, 2, H], F32, tag="xres_sb")
    nc.gpsimd.dma_start(xres_sb[:], xres.rearrange("(tt p) h -> p tt h", p=128))
    Bt_sb = cpool.tile([128, 4, S], BF16, tag="Bt_sb")
    nc.gpsimd.dma_start(Bt_sb[:], Bt.rearrange("(hh p) s -> p hh s", p=128))
    Dt_sb = cpool.tile([128, 4, H], BF16, tag="Dt_sb")
    nc.gpsimd.dma_start(Dt_sb[:], Dt.rearrange("(hh p) o -> p hh o", p=128))
    APOW_sb = cpool.tile([128, Q * S], BF16, tag="APOW_sb")
    nc.gpsimd.dma_start(APOW_sb[:], APOW)
    APQL_sb = cpool.tile([128, LZ * S], BF16, tag="APQL_sb")
    nc.gpsimd.dma_start(APQL_sb[:], APQL)
    GW_sb = cpool.tile([128, Q + Q * Q], BF16, tag="GW_sb")
    nc.gpsimd.dma_start(GW_sb[:], GW)
    if apply_gamma_beta:
        gb_sb = cpool.tile([128, 2, H], F32, tag="gb_sb")
        nc.gpsimd.dma_start(gb_sb[:], aps["gb"].rearrange("(u p) g h -> p g h", p=128))

    # ---- U = B @ x^T  (S x T) ---------------------------------------------
    U_ps = spp.tile([128, T], F32, tag="U_ps")
    for hh in range(4):
        nc.tensor.matmul(U_ps[:], lhsT=Bt_sb[:, hh, :], rhs=xT[hh][:],
                         start=(hh == 0), stop=(hh == 3))
    U_sb = cpool.tile([128, T], BF16, tag="U_sb")
    nc.vector.tensor_copy(U_sb[:], U_ps[:])
    U_r = U_sb.rearrange("s (j r) -> s r j", r=Q)      # [128, Q, NCH]

    # ---- chunk summaries R ------------------------------------------------
    R_ps = spp.tile([128, NCH], F32, tag="scan_ps")
    for r in range(Q):
        nc.tensor.matmul(R_ps[:], lhsT=APOW_sb[:, r * S:(r + 1) * S],
                         rhs=U_r[:, r, :], start=(r == 0), stop=(r == Q - 1))
    R_sb = cpool.tile([128, NCH], BF16, tag="R_sb")
    nc.vector.tensor_copy(R_sb[:], R_ps[:])

    # ---- boundary states Z (block-Toeplitz matmuls over lags) -------------
    Z_ps = spp.tile([128, NCH], F32, tag="scan_ps")
    for L in range(LZ):
        nc.tensor.matmul(Z_ps[:, L + 1:NCH], lhsT=APQL_sb[:, L * S:(L + 1) * S],
                         rhs=R_sb[:, 0:NCH - 1 - L],
                         start=(L == 0), stop=(L == LZ - 1))
    Z_sb = cpool.tile([128, NCH], BF16, tag="Z_sb")
    nc.gpsimd.memset(Z_sb[:], 0.0)
    nc.vector.tensor_copy(Z_sb[:, 1:NCH], Z_ps[:, 1:NCH])

    # ---- c^T = Z^T G + triangular intra-chunk term (j-part x i-free) ------
    c_psT = spp.tile([NCH, Q], F32, tag="scan_ps")
    nc.tensor.matmul(c_psT[:], lhsT=Z_sb[:], rhs=GW_sb[:, 0:Q],
                     start=True, stop=False)
    for k in range(Q):
        nc.tensor.matmul(c_psT[:], lhsT=U_r[:, k, :],
                         rhs=GW_sb[:, Q + k * Q:Q + (k + 1) * Q],
                         start=False, stop=(k == Q - 1))
    c_sbT = cpool.tile([NCH, Q], F32, tag="c_sbT")
    nc.vector.tensor_copy(c_sbT[:], c_psT[:])

    # ---- reshape c^T (j x i) -> per-row column via a DRAM bounce ----------
    # c^T partition-major flat order IS t = j*Q + i; output rows are
    # t in [256, 512), i.e. the last 256 values.
    c_dram = dpool.tile([NCH, Q], F32, tag="c_dram")
    nc.sync.dma_start(c_dram[:], c_sbT[:])
    c_col = cpool.tile([128, 2], F32, tag="c_col")
    c_lin = c_dram.rearrange("j i -> (j i)")[TOUT:].rearrange(
        "(n p) -> p n", p=128)
    nc.sync.dma_start(c_col[:], c_lin)

    # ---- xD + gelu + residual + layernorm ---------------------------------
    # Pass 1: matmuls, gelu(+c bias), residual add, batchnorm stats.
    y_sbs, mvs = [], []
    for tt2 in range(2):
        y_ps = ypp.tile([128, H], F32, tag="y_ps", name=f"y_ps{tt2}")
        for hh in range(4):
            nc.tensor.matmul(
                y_ps[:],
                lhsT=xT[hh][:, 256 + tt2 * 128:256 + (tt2 + 1) * 128],
                rhs=Dt_sb[:, hh, :], start=(hh == 0), stop=(hh == 3))
        g_sb = wpool.tile([128, H], F32, tag="g_sb", name=f"g_sb{tt2}")
        nc.scalar.activation(g_sb[:], y_ps[:], AF.Gelu,
                             bias=c_col[:, tt2:tt2 + 1], scale=1.0)
        y_sb = wpool.tile([128, H], F32, tag=f"y_sb{tt2}", name=f"y_sb{tt2}")
        nc.vector.tensor_add(y_sb[:], g_sb[:], xres_sb[:, tt2, :])
        st6 = wpool.tile([128, 6], F32, tag="st6", name=f"st6_{tt2}")
        nc.vector.bn_stats(st6[:], y_sb[:])
        mv = wpool.tile([128, 2], F32, tag=f"mv{tt2}", name=f"mv{tt2}")
        nc.vector.bn_aggr(mv[:], st6[:])
        y_sbs.append(y_sb)
        mvs.append(mv)

    # Dummy sqrt AFTER both gelus: pulls the sqrt ACT-table load off the
    # critical tail (it runs on ScalarE while the DVE does the bn stats).
    # +1e4 bias keeps the argument positive.
    sq_scr = wpool.tile([128, 1], F32, tag="sq_scr")
    nc.scalar.activation(sq_scr[:], y_sbs[1][:, 0:1], AF.Sqrt,
                         bias=big_sb[:], scale=1.0)

    # Pass 2: normalize and write out.
    for tt2 in range(2):
        y_sb, mv = y_sbs[tt2], mvs[tt2]
        sd = wpool.tile([128, 1], F32, tag=f"sd{tt2}", name=f"sd{tt2}")
        nc.scalar.activation(sd[:], mv[:, 1:2], AF.Sqrt, bias=eps_sb[:], scale=1.0)
        iv = wpool.tile([128, 1], F32, tag=f"iv{tt2}", name=f"iv{tt2}")
        nc.vector.reciprocal(iv[:], sd[:])
        o_sb = wpool.tile([128, H], F32, tag="o_sb", name=f"o_sb{tt2}")
        nc.vector.tensor_scalar(o_sb[:], y_sb[:], mv[:, 0:1], iv[:],
                                op0=mybir.AluOpType.subtract,
                                op1=mybir.AluOpType.mult)
        if apply_gamma_beta:
            nc.vector.tensor_mul(o_sb[:], o_sb[:], gb_sb[:, 0, :])
            nc.vector.tensor_add(o_sb[:], o_sb[:], gb_sb[:, 1, :])
        nc.sync.dma_start(yout[tt2 * 128:(tt2 + 1) * 128, :], o_sb[:])

    ctx.close()


def _build_program(apply_gamma_beta, LZ):
    nc = bacc.Bacc("TRN2", target_bir_lowering=False, debug=False,
                   enable_asserts=False, num_devices=NCORES)
    aps = {
        "xb": nc.dram_tensor("xb", (T, H), BF16, kind="ExternalInput").ap(),
        "xres": nc.dram_tensor("xres", (TOUT, H), F32, kind="ExternalInput").ap(),
        "Bt": nc.dram_tensor("Bt", (H, S), BF16, kind="ExternalInput").ap(),
        "Dt": nc.dram_tensor("Dt", (H, H), BF16, kind="ExternalInput").ap(),
        "APOW": nc.dram_tensor("APOW", (S, Q * S), BF16, kind="ExternalInput").ap(),
        "APQL": nc.dram_tensor("APQL", (S, LZ * S), BF16, kind="ExternalInput").ap(),
        "GW": nc.dram_tensor("GW", (S, Q + Q * Q), BF16, kind="ExternalInput").ap(),
        "yout": nc.dram_tensor("yout", (TOUT, H), F32, kind="ExternalOutput").ap(),
    }
    if apply_gamma_beta:
        aps["gb"] = nc.dram_tensor("gb", (128, 2, H), F32, kind="ExternalInput").ap()
    with tile.TileContext(nc) as tc:
        _emit(tc, aps, apply_gamma_beta, LZ)
    nc.compile()
    return nc


def _prepare_in_maps(x, A, Bm, Cm, D, gamma, beta, apply_gamma_beta):
    APOW, APQL, GW, LZ = _host_weights(A, Bm, Cm)
    base = {
        "Bt": np.ascontiguousarray(Bm.T).astype(BF16_NP),
        "Dt": np.ascontiguousarray(D.T).astype(BF16_NP),
        "APOW": APOW,
        "APQL": APQL,
        "GW": GW,
    }
    if apply_gamma_beta:
        gb = np.stack([np.broadcast_to(gamma, (128, H)),
                       np.broadcast_to(beta, (128, H))], axis=1)
        base["gb"] = np.ascontiguousarray(gb.astype(np.float32))
    in_maps = []
    for core in range(NCORES):
        b, half = core // 2, core % 2
        if half == 0:
            xb = np.concatenate(
                [np.zeros((TOUT, H), np.float32), x[b, :TOUT]], axis=0)
        else:
            xb = x[b]
        in_maps.append({
            **base,
            "xb": np.ascontiguousarray(xb).astype(BF16_NP),
            "xres": np.ascontiguousarray(xb[TOUT:]).astype(np.float32),
        })
    return in_maps, LZ


def _run(inputs, trace=False):
    x = np.asarray(inputs["x"], np.float32)
    A = np.asarray(inputs["A"], np.float32)
    Bm = np.asarray(inputs["B"], np.float32)
    Cm = np.asarray(inputs["C"], np.float32)
    D = np.asarray(inputs["D"], np.float32)
    gamma = np.asarray(inputs["gamma"], np.float32)
    beta = np.asarray(inputs["beta"], np.float32)

    apply_gamma_beta = not (np.all(gamma == 1.0) and np.all(beta == 0.0))
    in_maps, LZ = _prepare_in_maps(x, A, Bm, Cm, D, gamma, beta,
                                   apply_gamma_beta)
    nc = _build_program(apply_gamma_beta, LZ)
    res = bass_utils.run_bass_kernel_spmd(
        nc, in_maps, core_ids=list(range(NCORES)), trace=trace)
    y = np.empty((BSZ, T, H), np.float32)
    for core in range(NCORES):
        b, half = core // 2, core % 2
        y[b, half * TOUT:(half + 1) * TOUT, :] = res.results[core]["yout"]
    return y, res


def kernel(**inputs):
    y, _ = _run(inputs, trace=False)
    return y


def kernel_traced(**inputs):
    return _run(inputs, trace=True)
